# revision 6
# baseline (speedup 1.0000x reference)
"""FP8Linear kernel for Trainium2 (Bass/Tile), distributed over 8 NeuronCores.

Computation (matches the nn.Module reference):
    x:  [B=2, S=4096, K=4096] f32  -> x2d [M=8192, K]
    xq = tile_quant_dequant(x2d)    # per-row 1x64 chunks, fp8 e4m3fn round-trip
    wq = block_quant_dequant(w)     # 64x64 blocks of w [N=4096, K]
    out = f32(bf16(xq @ wq.T)) + bias  -> [B, S, N]

Distribution: 8x1 grid -- pure data-parallel over M. Each core takes 1024 x
rows + the full weight and computes a [1024, 4096] output shard. No
collectives, no DRAM scratch for operands.

Per-core dataflow (all-SBUF):
  - x quantized once into 8 resident xT units [128k, 33, 128m] bf16 via
    per-row 1x64 amax -> s2 = max(amax,1e-12)/224 -> q = fp8e4(x*rs2) ->
    dq = bf16(q*s2), then ONE SBUF->SBUF xbar DMA transpose per unit.
  - w quantized panel-by-panel (512 out_features ahead of the matmul sweeps)
    with 64x64 block scales (PE-transpose + DVE reduce + tiny DRAM bounce
    for the partition broadcast), same fp8 round-trip, transposed
    SBUF->SBUF into a double-buffered wT panel [128k, 33, 512n].
  - The 33rd k-slice carries the bias: xT slice = e0 (ones on partition 0),
    wT slice = bias row on partition 0 -> the PSUM accumulation adds b[n].
  - Sweeps: for panel, for m-tile: 33 bf16 matmuls accumulate [128,512]
    f32 in PSUM; ACT evacuates with a bf16 cast straight to the DRAM
    output (host casts to f32).
Engine split: DVE amax/scales/q-mult, GpSimd dq-mult, ACT evac + all
xbar transposes, Sync ring loads/stores/scale-bounce.
"""

import sys

sys.path.insert(0, "/opt/trn_rl_repo")

import numpy as np
from contextlib import ExitStack

import concourse.bass as bass
import concourse.mybir as mybir
import concourse.tile as tile
from concourse import bacc
from concourse.bass import ts
from concourse.masks import make_identity

P = 128
QT = 64  # quantization tile (1x64 for x, 64x64 for w)

# full-problem dims
B, S, K, N = 2, 4096, 4096, 4096
M = B * S
N_CORES = 8
M_SH = M // N_CORES  # 1024 rows of x per core

KB = K // P          # 32 k-slices of 128
KB1 = KB + 1         # +1 bias slice
KBF = K // QT        # 64 scale columns
NQ = 4               # quarters per 4096-wide row tile
KH = K // NQ         # 1024
KBQ = KBF // NQ      # 16 scale cols per quarter

N_PANEL = 512
PANELS = N // N_PANEL    # 8
MTILES = M_SH // P       # 8
WROWS_PER_PANEL = N_PANEL // P  # 4

F32 = mybir.dt.float32
BF16 = mybir.dt.bfloat16
FP8 = mybir.dt.float8e4


class Ctx:
    """Bag of pools / constants shared by the emit helpers."""


def _quant_rowtile(cx, nc, src, row0):
    """Load one [128, 4096] f32 row tile of `src` in quarters and compute the
    1x64 chunk amax. Returns (nat quarter tiles, amax [128, KBF] tile)."""
    nats = []
    a = cx.amax.tile([P, KBF], F32, tag="amax")
    for qq in range(NQ):
        nat = cx.nat.tile([P, KH], F32, tag="nat")
        nats.append(nat)
        nc.sync.dma_start(nat[:], src[row0 : row0 + P, qq * KH : (qq + 1) * KH])
        nc.vector.tensor_reduce(
            a[:, qq * KBQ : (qq + 1) * KBQ],
            nat[:].rearrange("p (c t) -> p c t", t=QT),
            axis=mybir.AxisListType.X, op=mybir.AluOpType.max,
            apply_absolute_value=True,
        )
    return nats, a


def _qdq(cx, nc, nats, dq, s2, rs2):
    """fp8 round-trip of the 4 loaded quarters into dq [128, 4096] bf16."""
    for qq in range(NQ):
        q = cx.q.tile([P, KH], FP8, tag="q")
        q_v = q[:].rearrange("p (c t) -> p c t", t=QT)
        nc.vector.tensor_tensor(
            q_v, nats[qq][:].rearrange("p (c t) -> p c t", t=QT),
            rs2[:, qq * KBQ : (qq + 1) * KBQ, None].to_broadcast((P, KBQ, QT)),
            op=mybir.AluOpType.mult,
        )
        nc.gpsimd.tensor_tensor(
            dq[:, qq * KH : (qq + 1) * KH].rearrange("p (c t) -> p c t", t=QT),
            q_v,
            s2[:, qq * KBQ : (qq + 1) * KBQ, None].to_broadcast((P, KBQ, QT)),
            op=mybir.AluOpType.mult,
        )


def _emit_x_unit(cx, nc, x, mi):
    """Quantize x rows [mi*128, +128) and transpose into resident xT unit."""
    xT = cx.xT.tile([P, KB1, P], BF16, tag="xT", bufs=MTILES, name=f"xT{mi}")
    cx.xT_units[mi] = xT
    # bias k-slice: e0 rows (ones on partition 0)
    nc.scalar.memzero(xT[:, KB, :])
    nc.vector.tensor_copy(xT[0:1, KB, :], cx.ones[:])

    nats, a = _quant_rowtile(cx, nc, x, mi * P)
    s2 = cx.scale.tile([P, KBF], F32, tag="s2x")
    rs2 = cx.scale.tile([P, KBF], F32, tag="rs2x")
    nc.vector.tensor_scalar(
        s2[:], a[:], 1e-12, 1.0 / 224.0,
        op0=mybir.AluOpType.max, op1=mybir.AluOpType.mult,
    )
    nc.vector.reciprocal(rs2[:], s2[:])
    dq = cx.dq.tile([P, K], BF16, tag="dq")
    _qdq(cx, nc, nats, dq, s2, rs2)
    nc.scalar.dma_start(xT[:, 0:KB, :], dq[:], transpose=True)


def _emit_w_rowtile(cx, nc, w, wt, ns, wTp):
    """Quantize w rows [wt*128, +128) with 64x64 block scales and transpose
    into column block ns of the panel tile wTp."""
    nats, a = _quant_rowtile(cx, nc, w, wt * P)
    # 64x64 block amax: PE-transpose the [128, KBF] chunk-amax, reduce the
    # two 64-row groups, rebroadcast through a tiny DRAM bounce.
    at_ps = cx.tpsum.tile([KBF, P], F32, tag="at_ps")
    nc.tensor.transpose(at_ps[:], a[:], cx.ident_f32[:])
    r = cx.amax.tile([KBF, 2], F32, tag="r_blk")
    nc.vector.tensor_reduce(
        r[:], at_ps[:].rearrange("p (g t) -> p g t", t=QT),
        axis=mybir.AxisListType.X, op=mybir.AluOpType.max,
    )
    s2blk = cx.amax.tile([KBF, 2], F32, tag="s2blk")
    nc.vector.tensor_scalar(
        s2blk[:], r[:], 1e-12, 1.0 / 224.0,
        op0=mybir.AluOpType.max, op1=mybir.AluOpType.mult,
    )
    s2 = cx.scale.tile([P, KBF], F32, tag="s2w")
    rs2 = cx.scale.tile([P, KBF], F32, tag="rs2w")
    for nb in (0, 1):
        srow = cx.dram_small.tile([1, KBF], F32, tag="srow")
        nc.sync.dma_start(srow[:], s2blk[:, nb : nb + 1])
        nc.sync.dma_start(
            s2[nb * QT : (nb + 1) * QT, :],
            srow[:].to_broadcast((QT, KBF)),
        )
    nc.vector.reciprocal(rs2[:], s2[:])
    dq = cx.dq.tile([P, K], BF16, tag="dq")
    _qdq(cx, nc, nats, dq, s2, rs2)
    nc.scalar.dma_start(wTp[:, 0:KB, ts(ns, P)], dq[:], transpose=True)


def _emit_w_panel(cx, nc, w, pn):
    wTp = cx.wT.tile([P, KB1, N_PANEL], BF16, tag="wT")
    # bias k-slice: b[pn*512 : +512] on partition 0, zero elsewhere
    nc.scalar.memzero(wTp[:, KB, :])
    nc.sync.dma_start(
        wTp[0:1, KB, :],
        cx.bias_bf16[pn * WROWS_PER_PANEL : (pn + 1) * WROWS_PER_PANEL, :],
    )
    for i in range(WROWS_PER_PANEL):
        _emit_w_rowtile(cx, nc, w, pn * WROWS_PER_PANEL + i, i, wTp)
    return wTp


def _emit_sweep(cx, nc, out, mi, pn, wTp):
    ps = cx.mpsum.tile([P, N_PANEL], F32, tag="mpsum")
    xT = cx.xT_units[mi]
    for kb in range(KB1):
        nc.tensor.matmul(
            ps[:], xT[:, kb, :], wTp[:, kb, :],
            start=(kb == 0), stop=(kb == KB1 - 1),
        )
    ev = cx.evac.tile([P, N_PANEL], BF16, tag="evac")
    nc.scalar.copy(ev[:], ps[:])
    nc.sync.dma_start(out[ts(mi, P), ts(pn, N_PANEL)], ev[:])


def fp8_linear_core_kernel(tc, out, x, w, b):
    """Per-core: out [M_SH, N] bf16 = bf16(xq @ wq.T + b).
    x [M_SH, K] f32, w [N, K] f32, b [32, 128] f32 (= bias reshaped)."""
    nc = tc.nc
    ctx = tc.ctx

    cx = Ctx()
    cx.nat = ctx.enter_context(tc.tile_pool(name="nat", bufs=6))
    cx.q = ctx.enter_context(tc.tile_pool(name="q", bufs=3))
    cx.dq = ctx.enter_context(tc.tile_pool(name="dq", bufs=3))
    cx.amax = ctx.enter_context(tc.tile_pool(name="amax", bufs=3))
    cx.scale = ctx.enter_context(tc.tile_pool(name="scale", bufs=2))
    cx.xT = ctx.enter_context(tc.tile_pool(name="xT", bufs=MTILES))
    cx.wT = ctx.enter_context(tc.tile_pool(name="wT", bufs=2))
    cx.mpsum = ctx.enter_context(tc.tile_pool(name="mpsum", bufs=6, space="PSUM"))
    cx.tpsum = ctx.enter_context(tc.tile_pool(name="tpsum", bufs=2, space="PSUM"))
    cx.evac = ctx.enter_context(tc.tile_pool(name="evac", bufs=4))
    cx.const = ctx.enter_context(tc.tile_pool(name="const", bufs=1))
    cx.dram_small = ctx.enter_context(
        tc.tile_pool(name="scratch_s", bufs=8, space="DRAM")
    )
    cx.xT_units = [None] * MTILES

    cx.ident_f32 = cx.const.tile([P, P], F32, tag="ident")
    make_identity(nc, cx.ident_f32)

    # ones row [1, 128] bf16 for the bias k-slice of xT
    cx.ones = cx.const.tile([1, P], BF16, tag="ones")
    nc.scalar.memzero(cx.ones[:])
    nc.scalar.add(cx.ones[:], cx.ones[:], 1.0)

    # bias as bf16 in DRAM scratch, laid out [32, 128] row-major = b[4096]
    bt = cx.const.tile([32, P], F32, tag="bt")
    nc.sync.dma_start(bt[:], b)
    btb = cx.const.tile([32, P], BF16, tag="btb")
    nc.vector.tensor_copy(btb[:], bt[:])
    bias_dram = cx.dram_small.tile([32, P], BF16, tag="bias_dram")
    nc.gpsimd.dma_start(bias_dram[:], btb[:])
    cx.bias_bf16 = bias_dram

    # ---- production + sweeps ----
    # Panel 0 first (PE can start as soon as wT0 + xT0 are ready), x units
    # next with panel-1 row tiles woven in, then steady-state: emit panel
    # p's production just before its sweeps so it is produced one panel
    # ahead while panel p-1 sweeps.
    wT0 = _emit_w_panel(cx, nc, w, 0)

    w1_tiles = []
    wT1 = cx.wT.tile([P, KB1, N_PANEL], BF16, tag="wT")
    nc.scalar.memzero(wT1[:, KB, :])
    nc.sync.dma_start(wT1[0:1, KB, :], cx.bias_bf16[WROWS_PER_PANEL : 2 * WROWS_PER_PANEL, :])

    for mi in range(MTILES):
        _emit_x_unit(cx, nc, x, mi)
        # weave w panel-1 row tiles early (after x units 0..3) so wT1 is
        # complete by the time the panel-0 sweeps finish chasing x
        if mi < WROWS_PER_PANEL:
            _emit_w_rowtile(cx, nc, w, WROWS_PER_PANEL + mi, mi, wT1)
        _emit_sweep(cx, nc, out, mi, 0, wT0)
    for mi in range(MTILES):
        _emit_sweep(cx, nc, out, mi, 1, wT1)

    for pn in range(2, PANELS):
        wTp = _emit_w_panel(cx, nc, w, pn)
        for mi in range(MTILES):
            _emit_sweep(cx, nc, out, mi, pn, wTp)


def build_core_bass():
    nc = bacc.Bacc(
        "TRN2", target_bir_lowering=False, debug=False, num_devices=N_CORES
    )
    x = nc.dram_tensor("x", [M_SH, K], F32, kind="ExternalInput").ap()
    w = nc.dram_tensor("w", [N, K], F32, kind="ExternalInput").ap()
    b = nc.dram_tensor("b", [32, P], F32, kind="ExternalInput").ap()
    out = nc.dram_tensor("out", [M_SH, N], BF16, kind="ExternalOutput").ap()
    with tile.TileContext(nc) as tc:
        with ExitStack() as stack:
            tc.ctx = stack
            fp8_linear_core_kernel(tc, out, x, w, b)
    nc.compile()
    return nc


_NC_CACHE = []


def _get_nc():
    if not _NC_CACHE:
        _NC_CACHE.append(build_core_bass())
    return _NC_CACHE[0]


def kernel(x, weight, bias):
    """Full-problem entry point: x [2,4096,4096] f32, weight [4096,4096] f32,
    bias [4096] f32 -> [2,4096,4096] f32."""
    from concourse.bass_utils import run_bass_kernel_spmd

    x2d = np.ascontiguousarray(x.reshape(M, K), dtype=np.float32)
    weight = np.ascontiguousarray(weight, dtype=np.float32)
    b32 = np.ascontiguousarray(bias.reshape(32, P), dtype=np.float32)

    nc = _get_nc()

    in_maps = []
    for core in range(N_CORES):
        in_maps.append(
            {
                "x": np.ascontiguousarray(x2d[core * M_SH : (core + 1) * M_SH]),
                "w": weight,
                "b": b32,
            }
        )

    res = run_bass_kernel_spmd(nc, in_maps, core_ids=list(range(N_CORES)))
    global LAST_EXEC_TIME_NS
    LAST_EXEC_TIME_NS = res.exec_time_ns

    out = np.empty((M, N), dtype=np.float32)
    for core in range(N_CORES):
        out[core * M_SH : (core + 1) * M_SH] = np.asarray(
            res.results[core]["out"]
        ).astype(np.float32)
    return out.reshape(B, S, N)


# revision 7
# speedup vs baseline: 1.1496x; 1.1496x over previous
"""FP8Linear kernel for Trainium2 (Bass/Tile), distributed over 8 NeuronCores.

Computation (matches the nn.Module reference):
    x:  [B=2, S=4096, K=4096] f32  -> x2d [M=8192, K]
    xq = tile_quant_dequant(x2d)    # per-row 1x64 chunks, fp8 e4m3fn round-trip
    wq = block_quant_dequant(w)     # 64x64 blocks of w [N=4096, K]
    out = f32(bf16(xq @ wq.T)) + bias  -> [B, S, N]

Distribution: 8x1 grid -- pure data-parallel over M. Each core takes 1024 x
rows + the full weight and computes a [1024, 4096] output shard. No
collectives, no DRAM scratch for operands.

Per-core dataflow (all-SBUF):
  - x quantized once into 8 resident xT units [128k, 33, 128m] bf16 via
    per-row 1x64 amax -> s2 = max(amax,1e-12)/224 -> q = fp8e4(x*rs2) ->
    dq = bf16(q*s2), then ONE SBUF->SBUF xbar DMA transpose per unit.
  - w quantized panel-by-panel (512 out_features ahead of the matmul sweeps)
    with 64x64 block scales (PE-transpose + DVE reduce + tiny DRAM bounce
    for the partition broadcast), same fp8 round-trip, transposed
    SBUF->SBUF into a double-buffered wT panel [128k, 33, 512n].
  - The 33rd k-slice carries the bias: xT slice = e0 (ones on partition 0),
    wT slice = bias row on partition 0 -> the PSUM accumulation adds b[n].
  - Sweeps: for panel, for m-tile: 33 bf16 matmuls accumulate [128,512]
    f32 in PSUM; ACT evacuates with a bf16 cast straight to the DRAM
    output (host casts to f32).
Engine split: DVE amax/scales/q-mult, GpSimd dq-mult, ACT evac + all
xbar transposes, Sync ring loads/stores/scale-bounce.
"""

import sys

sys.path.insert(0, "/opt/trn_rl_repo")

import numpy as np
from contextlib import ExitStack

import concourse.bass as bass
import concourse.mybir as mybir
import concourse.tile as tile
from concourse import bacc
from concourse.bass import ts
from concourse.masks import make_identity

P = 128
QT = 64  # quantization tile (1x64 for x, 64x64 for w)

# full-problem dims
B, S, K, N = 2, 4096, 4096, 4096
M = B * S
N_CORES = 8
M_SH = M // N_CORES  # 1024 rows of x per core

KB = K // P          # 32 k-slices of 128
KB1 = KB + 1         # +1 bias slice
KBF = K // QT        # 64 scale columns
NQ = 4               # quarters per 4096-wide row tile
KH = K // NQ         # 1024
KBQ = KBF // NQ      # 16 scale cols per quarter

N_PANEL = 512
PANELS = N // N_PANEL    # 8
MTILES = M_SH // P       # 8
WROWS_PER_PANEL = N_PANEL // P  # 4

F32 = mybir.dt.float32
BF16 = mybir.dt.bfloat16
FP8 = mybir.dt.float8e4


class Ctx:
    """Bag of pools / constants shared by the emit helpers."""


def _quant_rowtile(cx, nc, src, row0):
    """Load one [128, 4096] f32 row tile of `src` in quarters and compute the
    1x64 chunk amax. Returns (nat quarter tiles, amax [128, KBF] tile)."""
    nats = []
    a = cx.amax.tile([P, KBF], F32, tag="amax")
    for qq in range(NQ):
        nat = cx.nat.tile([P, KH], F32, tag="nat")
        nats.append(nat)
        nc.sync.dma_start(nat[:], src[row0 : row0 + P, qq * KH : (qq + 1) * KH])
        nc.vector.tensor_reduce(
            a[:, qq * KBQ : (qq + 1) * KBQ],
            nat[:].rearrange("p (c t) -> p c t", t=QT),
            axis=mybir.AxisListType.X, op=mybir.AluOpType.max,
            apply_absolute_value=True,
        )
    return nats, a


def _qdq(cx, nc, nats, dq, s2, rs2):
    """fp8 round-trip of the 4 loaded quarters into dq [128, 4096] bf16."""
    for qq in range(NQ):
        q = cx.q.tile([P, KH], FP8, tag="q")
        q_v = q[:].rearrange("p (c t) -> p c t", t=QT)
        nc.vector.tensor_tensor(
            q_v, nats[qq][:].rearrange("p (c t) -> p c t", t=QT),
            rs2[:, qq * KBQ : (qq + 1) * KBQ, None].to_broadcast((P, KBQ, QT)),
            op=mybir.AluOpType.mult,
        )
        nc.gpsimd.tensor_tensor(
            dq[:, qq * KH : (qq + 1) * KH].rearrange("p (c t) -> p c t", t=QT),
            q_v,
            s2[:, qq * KBQ : (qq + 1) * KBQ, None].to_broadcast((P, KBQ, QT)),
            op=mybir.AluOpType.mult,
        )


def _emit_x_unit(cx, nc, x, mi):
    """Quantize x rows [mi*128, +128) and transpose into resident xT unit."""
    xT = cx.xT.tile([P, KB1, P], BF16, tag="xT", bufs=MTILES, name=f"xT{mi}")
    cx.xT_units[mi] = xT
    # bias k-slice: e0 rows (ones on partition 0)
    nc.scalar.memzero(xT[:, KB, :])
    nc.vector.tensor_copy(xT[0:1, KB, :], cx.ones[:])

    nats, a = _quant_rowtile(cx, nc, x, mi * P)
    s2 = cx.scale.tile([P, KBF], F32, tag="s2x")
    rs2 = cx.scale.tile([P, KBF], F32, tag="rs2x")
    nc.vector.tensor_scalar(
        s2[:], a[:], 1e-12, 1.0 / 224.0,
        op0=mybir.AluOpType.max, op1=mybir.AluOpType.mult,
    )
    nc.vector.reciprocal(rs2[:], s2[:])
    dq = cx.dq.tile([P, K], BF16, tag="dq")
    _qdq(cx, nc, nats, dq, s2, rs2)
    nc.scalar.dma_start(xT[:, 0:KB, :], dq[:], transpose=True)


def _emit_w_rowtile(cx, nc, w, wt, ns, wTp):
    """Quantize w rows [wt*128, +128) with 64x64 block scales and transpose
    into column block ns of the panel tile wTp."""
    nats, a = _quant_rowtile(cx, nc, w, wt * P)
    # 64x64 block amax: PE-transpose the [128, KBF] chunk-amax, reduce the
    # two 64-row groups, rebroadcast through a tiny DRAM bounce.
    at_ps = cx.tpsum.tile([KBF, P], F32, tag="at_ps")
    nc.tensor.transpose(at_ps[:], a[:], cx.ident_f32[:])
    r = cx.amax.tile([KBF, 2], F32, tag="r_blk")
    nc.vector.tensor_reduce(
        r[:], at_ps[:].rearrange("p (g t) -> p g t", t=QT),
        axis=mybir.AxisListType.X, op=mybir.AluOpType.max,
    )
    s2blk = cx.amax.tile([KBF, 2], F32, tag="s2blk")
    nc.vector.tensor_scalar(
        s2blk[:], r[:], 1e-12, 1.0 / 224.0,
        op0=mybir.AluOpType.max, op1=mybir.AluOpType.mult,
    )
    s2 = cx.scale.tile([P, KBF], F32, tag="s2w")
    rs2 = cx.scale.tile([P, KBF], F32, tag="rs2w")
    for nb in (0, 1):
        srow = cx.dram_small.tile([1, KBF], F32, tag="srow")
        nc.sync.dma_start(srow[:], s2blk[:, nb : nb + 1])
        nc.sync.dma_start(
            s2[nb * QT : (nb + 1) * QT, :],
            srow[:].to_broadcast((QT, KBF)),
        )
    nc.vector.reciprocal(rs2[:], s2[:])
    dq = cx.dq.tile([P, K], BF16, tag="dq")
    _qdq(cx, nc, nats, dq, s2, rs2)
    nc.scalar.dma_start(wTp[:, 0:KB, ts(ns, P)], dq[:], transpose=True)


def _emit_w_panel(cx, nc, w, pn):
    wTp = cx.wT.tile([P, KB1, N_PANEL], BF16, tag="wT")
    # bias k-slice: b[pn*512 : +512] on partition 0, zero elsewhere
    nc.scalar.memzero(wTp[:, KB, :])
    nc.sync.dma_start(
        wTp[0:1, KB, :],
        cx.bias_bf16[pn * WROWS_PER_PANEL : (pn + 1) * WROWS_PER_PANEL, :],
    )
    for i in range(WROWS_PER_PANEL):
        _emit_w_rowtile(cx, nc, w, pn * WROWS_PER_PANEL + i, i, wTp)
    return wTp


def _emit_sweep(cx, nc, out, mi, pn, wTp):
    ps = cx.mpsum.tile([P, N_PANEL], F32, tag="mpsum")
    xT = cx.xT_units[mi]
    for kb in range(KB1):
        nc.tensor.matmul(
            ps[:], xT[:, kb, :], wTp[:, kb, :],
            start=(kb == 0), stop=(kb == KB1 - 1),
        )
    ev = cx.evac.tile([P, N_PANEL], BF16, tag="evac")
    nc.scalar.copy(ev[:], ps[:])
    nc.sync.dma_start(out[ts(mi, P), ts(pn, N_PANEL)], ev[:])


def fp8_linear_core_kernel(tc, out, x, w, b):
    """Per-core: out [M_SH, N] bf16 = bf16(xq @ wq.T + b).
    x [M_SH, K] f32, w [N, K] f32, b [32, 128] f32 (= bias reshaped)."""
    nc = tc.nc
    ctx = tc.ctx

    cx = Ctx()
    cx.nat = ctx.enter_context(tc.tile_pool(name="nat", bufs=6))
    cx.q = ctx.enter_context(tc.tile_pool(name="q", bufs=3))
    cx.dq = ctx.enter_context(tc.tile_pool(name="dq", bufs=3))
    cx.amax = ctx.enter_context(tc.tile_pool(name="amax", bufs=3))
    cx.scale = ctx.enter_context(tc.tile_pool(name="scale", bufs=2))
    cx.xT = ctx.enter_context(tc.tile_pool(name="xT", bufs=MTILES))
    cx.wT = ctx.enter_context(tc.tile_pool(name="wT", bufs=2))
    cx.mpsum = ctx.enter_context(tc.tile_pool(name="mpsum", bufs=6, space="PSUM"))
    cx.tpsum = ctx.enter_context(tc.tile_pool(name="tpsum", bufs=2, space="PSUM"))
    cx.evac = ctx.enter_context(tc.tile_pool(name="evac", bufs=4))
    cx.const = ctx.enter_context(tc.tile_pool(name="const", bufs=1))
    cx.dram_small = ctx.enter_context(
        tc.tile_pool(name="scratch_s", bufs=8, space="DRAM")
    )
    cx.xT_units = [None] * MTILES

    cx.ident_f32 = cx.const.tile([P, P], F32, tag="ident")
    make_identity(nc, cx.ident_f32)

    # ones row [1, 128] bf16 for the bias k-slice of xT
    cx.ones = cx.const.tile([1, P], BF16, tag="ones")
    nc.scalar.memzero(cx.ones[:])
    nc.scalar.add(cx.ones[:], cx.ones[:], 1.0)

    # bias as bf16 in DRAM scratch, laid out [32, 128] row-major = b[4096]
    bt = cx.const.tile([32, P], F32, tag="bt")
    nc.sync.dma_start(bt[:], b)
    btb = cx.const.tile([32, P], BF16, tag="btb")
    nc.vector.tensor_copy(btb[:], bt[:])
    bias_dram = cx.dram_small.tile([32, P], BF16, tag="bias_dram")
    nc.gpsimd.dma_start(bias_dram[:], btb[:])
    cx.bias_bf16 = bias_dram

    # ---- production + sweeps ----
    # Panel 0 first (PE can start as soon as wT0 + xT0 are ready), x units
    # next with panel-1 row tiles woven in, then steady-state: emit panel
    # p's production just before its sweeps so it is produced one panel
    # ahead while panel p-1 sweeps.
    wT0 = _emit_w_panel(cx, nc, w, 0)

    w1_tiles = []
    wT1 = cx.wT.tile([P, KB1, N_PANEL], BF16, tag="wT")
    nc.scalar.memzero(wT1[:, KB, :])
    nc.sync.dma_start(wT1[0:1, KB, :], cx.bias_bf16[WROWS_PER_PANEL : 2 * WROWS_PER_PANEL, :])

    for mi in range(MTILES):
        _emit_x_unit(cx, nc, x, mi)
        # weave w panel-1 row tiles early (after x units 0..3) so wT1 is
        # complete by the time the panel-0 sweeps finish chasing x
        if mi < WROWS_PER_PANEL:
            _emit_w_rowtile(cx, nc, w, WROWS_PER_PANEL + mi, mi, wT1)
        _emit_sweep(cx, nc, out, mi, 0, wT0)

    # Steady state: weave panel p+1's row-tile production between panel p's
    # sweeps so the PE FIFO holds [scaleT(p+1,r), s(p,mi), ...] -- next-panel
    # production never queues behind a full panel of matmuls on any engine.
    wTs = {1: wT1}
    for pn in range(1, PANELS):
        nxt = pn + 1
        if nxt < PANELS:
            wTn = cx.wT.tile([P, KB1, N_PANEL], BF16, tag="wT")
            nc.scalar.memzero(wTn[:, KB, :])
            nc.sync.dma_start(
                wTn[0:1, KB, :],
                cx.bias_bf16[nxt * WROWS_PER_PANEL : (nxt + 1) * WROWS_PER_PANEL, :],
            )
            wTs[nxt] = wTn
        for mi in range(MTILES):
            if nxt < PANELS and mi < WROWS_PER_PANEL:
                _emit_w_rowtile(cx, nc, w, nxt * WROWS_PER_PANEL + mi, mi, wTs[nxt])
            _emit_sweep(cx, nc, out, mi, pn, wTs[pn])


def build_core_bass():
    nc = bacc.Bacc(
        "TRN2", target_bir_lowering=False, debug=False, num_devices=N_CORES
    )
    x = nc.dram_tensor("x", [M_SH, K], F32, kind="ExternalInput").ap()
    w = nc.dram_tensor("w", [N, K], F32, kind="ExternalInput").ap()
    b = nc.dram_tensor("b", [32, P], F32, kind="ExternalInput").ap()
    out = nc.dram_tensor("out", [M_SH, N], BF16, kind="ExternalOutput").ap()
    with tile.TileContext(nc) as tc:
        with ExitStack() as stack:
            tc.ctx = stack
            fp8_linear_core_kernel(tc, out, x, w, b)
    nc.compile()
    return nc


_NC_CACHE = []


def _get_nc():
    if not _NC_CACHE:
        _NC_CACHE.append(build_core_bass())
    return _NC_CACHE[0]


def kernel(x, weight, bias):
    """Full-problem entry point: x [2,4096,4096] f32, weight [4096,4096] f32,
    bias [4096] f32 -> [2,4096,4096] f32."""
    from concourse.bass_utils import run_bass_kernel_spmd

    x2d = np.ascontiguousarray(x.reshape(M, K), dtype=np.float32)
    weight = np.ascontiguousarray(weight, dtype=np.float32)
    b32 = np.ascontiguousarray(bias.reshape(32, P), dtype=np.float32)

    nc = _get_nc()

    in_maps = []
    for core in range(N_CORES):
        in_maps.append(
            {
                "x": np.ascontiguousarray(x2d[core * M_SH : (core + 1) * M_SH]),
                "w": weight,
                "b": b32,
            }
        )

    res = run_bass_kernel_spmd(nc, in_maps, core_ids=list(range(N_CORES)))
    global LAST_EXEC_TIME_NS
    LAST_EXEC_TIME_NS = res.exec_time_ns

    out = np.empty((M, N), dtype=np.float32)
    for core in range(N_CORES):
        out[core * M_SH : (core + 1) * M_SH] = np.asarray(
            res.results[core]["out"]
        ).astype(np.float32)
    return out.reshape(B, S, N)


# revision 9
# speedup vs baseline: 1.5276x; 1.3288x over previous
"""FP8Linear kernel for Trainium2 (Bass/Tile), distributed over 8 NeuronCores.

Computation (matches the nn.Module reference):
    x:  [B=2, S=4096, K=4096] f32  -> x2d [M=8192, K]
    xq = tile_quant_dequant(x2d)    # per-row 1x64 chunks, fp8 e4m3fn round-trip
    wq = block_quant_dequant(w)     # 64x64 blocks of w [N=4096, K]
    out = f32(bf16(xq @ wq.T)) + bias  -> [B, S, N]

Distribution: 8x1 grid -- pure data-parallel over M. Each core takes 1024 x
rows + the full weight and computes a [1024, 4096] output shard. No
collectives, no DRAM operand scratch.

Per-core dataflow (all-SBUF):
  - x quantized once into 8 resident xT units [128k, 33, 128m] bf16 via
    per-row 1x64 amax -> s2 = max(amax,1e-12)/224 -> q = fp8e4(x*rs2) ->
    dq = bf16(q*s2), then ONE SBUF->SBUF xbar DMA transpose per unit.
  - w quantized panel-by-panel (512 out_features ahead of the matmul
    sweeps) with 64x64 block scales. The scale stage (amax, PE-transpose,
    group reduce, s2 and 1/s2 on the tiny [64,2] tile, DRAM-bounce
    partition broadcast) runs ONE ROW TILE AHEAD of the q/dq stage so the
    bounce latency hides under a sweep.
  - The 33rd k-slice carries the bias: xT slice = e0 (ones on partition 0),
    wT slice = bias row on partition 0 -> the PSUM accumulation adds b[n].
  - Sweeps: for panel, for m-tile: 33 bf16 matmuls accumulate [128,512]
    f32 in PSUM; ACT evacuates with a bf16 cast straight to the DRAM
    output (host casts to f32).
Engine split: DVE amax/scales/q-mult, GpSimd dq-mult, ACT evac + all
xbar transposes, Sync ring loads/stores/scale-bounce.
"""

import sys

sys.path.insert(0, "/opt/trn_rl_repo")

import numpy as np
from contextlib import ExitStack

import concourse.bass as bass
import concourse.mybir as mybir
import concourse.tile as tile
from concourse import bacc
from concourse.bass import ts
from concourse.masks import make_identity

P = 128
QT = 64  # quantization tile (1x64 for x, 64x64 for w)

# full-problem dims
B, S, K, N = 2, 4096, 4096, 4096
M = B * S
N_CORES = 8
M_SH = M // N_CORES  # 1024 rows of x per core

KB = K // P          # 32 k-slices of 128
KB1 = KB + 1         # +1 bias slice
KBF = K // QT        # 64 scale columns
NH = 2               # halves per 4096-wide row tile
KH = K // NH         # 2048
KBH = KBF // NH      # 32 scale cols per half

N_PANEL = 512
PANELS = N // N_PANEL    # 8
MTILES = M_SH // P       # 8
WR = N_PANEL // P        # 4 w row tiles per panel

F32 = mybir.dt.float32
BF16 = mybir.dt.bfloat16
FP8 = mybir.dt.float8e4


class Ctx:
    """Bag of pools / constants shared by the emit helpers."""


def _load_amax(cx, nc, src, row0):
    """Load one [128, 4096] f32 row tile of `src` in halves and compute the
    1x64 chunk amax. Returns (nat half tiles, amax [128, KBF] tile)."""
    nats = []
    a = cx.amax.tile([P, KBF], F32, tag="amax")
    for hh in range(NH):
        nat = cx.nat.tile([P, KH], F32, tag="nat")
        nats.append(nat)
        nc.sync.dma_start(nat[:], src[row0 : row0 + P, hh * KH : (hh + 1) * KH])
        nc.vector.tensor_reduce(
            a[:, hh * KBH : (hh + 1) * KBH],
            nat[:].rearrange("p (c t) -> p c t", t=QT),
            axis=mybir.AxisListType.X, op=mybir.AluOpType.max,
            apply_absolute_value=True,
        )
    return nats, a


def _qdq(cx, nc, nats, dq, s2ap, rs2ap):
    """fp8 round-trip of the loaded halves into dq [128, 4096] bf16.
    s2ap/rs2ap: [P, KBF] access patterns (possibly strided)."""
    for hh in range(NH):
        q = cx.q.tile([P, KH], FP8, tag="q")
        q_v = q[:].rearrange("p (c t) -> p c t", t=QT)
        nc.vector.tensor_tensor(
            q_v, nats[hh][:].rearrange("p (c t) -> p c t", t=QT),
            rs2ap[:, hh * KBH : (hh + 1) * KBH, None].to_broadcast((P, KBH, QT)),
            op=mybir.AluOpType.mult,
        )
        nc.gpsimd.tensor_tensor(
            dq[:, hh * KH : (hh + 1) * KH].rearrange("p (c t) -> p c t", t=QT),
            q_v,
            s2ap[:, hh * KBH : (hh + 1) * KBH, None].to_broadcast((P, KBH, QT)),
            op=mybir.AluOpType.mult,
        )


def _emit_x_unit(cx, nc, x, mi):
    """Quantize x rows [mi*128, +128) and transpose into resident xT unit."""
    xT = cx.xT.tile([P, KB1, P], BF16, tag="xT", bufs=MTILES, name=f"xT{mi}")
    cx.xT_units[mi] = xT
    # bias k-slice: e0 rows (ones on partition 0)
    nc.scalar.memzero(xT[:, KB, :])
    nc.vector.tensor_copy(xT[0:1, KB, :], cx.ones[:])

    nats, a = _load_amax(cx, nc, x, mi * P)
    s2 = cx.scale.tile([P, KBF], F32, tag="s2x")
    rs2 = cx.scale.tile([P, KBF], F32, tag="rs2x")
    nc.vector.tensor_scalar(
        s2[:], a[:], 1e-12, 1.0 / 224.0,
        op0=mybir.AluOpType.max, op1=mybir.AluOpType.mult,
    )
    nc.vector.reciprocal(rs2[:], s2[:])
    dq = cx.dq.tile([P, K], BF16, tag="dq")
    _qdq(cx, nc, nats, dq, s2[:], rs2[:])
    nc.scalar.dma_start(xT[:, 0:KB, :], dq[:], transpose=True)


def _w_scale_stage(cx, nc, w, wt):
    """Stage A for w row tile wt: load + chunk amax + 64x64 block scales,
    with s2 AND 1/s2 bounced through DRAM into an interleaved [P, KBF, 2]
    broadcast tile. Returns (nats, srs tile) for the deferred qdq stage."""
    nats, a = _load_amax(cx, nc, w, wt * P)
    at_ps = cx.tpsum.tile([KBF, P], F32, tag="at_ps")
    nc.tensor.transpose(at_ps[:], a[:], cx.ident_f32[:])
    r = cx.amax.tile([KBF, 2], F32, tag="r_blk")
    nc.vector.tensor_reduce(
        r[:], at_ps[:].rearrange("p (g t) -> p g t", t=QT),
        axis=mybir.AxisListType.X, op=mybir.AluOpType.max,
    )
    # sblk[:, nb, 0] = s2 = max(amax,eps)/224 ; sblk[:, nb, 1] = 1/s2
    sblk = cx.amax.tile([KBF, 2, 2], F32, tag="sblk")
    nc.vector.tensor_scalar(
        sblk[:, :, 0], r[:], 1e-12, 1.0 / 224.0,
        op0=mybir.AluOpType.max, op1=mybir.AluOpType.mult,
    )
    nc.vector.reciprocal(sblk[:, :, 1], sblk[:, :, 0])
    srs = cx.scale.tile([P, KBF, 2], F32, tag="srsw")
    for nb in (0, 1):
        srow = cx.dram_small.tile([1, 2 * KBF], F32, tag="srow")
        nc.sync.dma_start(srow[:], sblk[:, nb, :])
        nc.sync.dma_start(
            srs[nb * QT : (nb + 1) * QT, :, :],
            srow[:].rearrange("o (k j) -> o k j", j=2).to_broadcast((QT, KBF, 2)),
        )
    return nats, srs


def _w_qdq_stage(cx, nc, staged, ns, wTp):
    """Stage B: fp8 round-trip + transpose into column block ns of wTp."""
    nats, srs = staged
    dq = cx.dq.tile([P, K], BF16, tag="dq")
    _qdq(cx, nc, nats, dq, srs[:, :, 0], srs[:, :, 1])
    nc.scalar.dma_start(wTp[:, 0:KB, ts(ns, P)], dq[:], transpose=True)


def _alloc_wT(cx, nc, pn):
    wTp = cx.wT.tile([P, KB1, N_PANEL], BF16, tag="wT")
    nc.scalar.memzero(wTp[:, KB, :])
    nc.sync.dma_start(
        wTp[0:1, KB, :],
        cx.bias_bf16[pn * WR : (pn + 1) * WR, :],
    )
    return wTp


def _emit_sweep(cx, nc, out, mi, pn, wTp):
    ps = cx.mpsum.tile([P, N_PANEL], F32, tag="mpsum")
    xT = cx.xT_units[mi]
    for kb in range(KB1):
        nc.tensor.matmul(
            ps[:], xT[:, kb, :], wTp[:, kb, :],
            start=(kb == 0), stop=(kb == KB1 - 1),
        )
    ev = cx.evac.tile([P, N_PANEL], BF16, tag="evac")
    nc.scalar.copy(ev[:], ps[:])
    nc.sync.dma_start(out[ts(mi, P), ts(pn, N_PANEL)], ev[:])


def fp8_linear_core_kernel(tc, out, x, w, b):
    """Per-core: out [M_SH, N] bf16 = bf16(xq @ wq.T + b).
    x [M_SH, K] f32, w [N, K] f32, b [32, 128] f32 (= bias reshaped)."""
    nc = tc.nc
    ctx = tc.ctx

    cx = Ctx()
    cx.nat = ctx.enter_context(tc.tile_pool(name="nat", bufs=5))
    cx.q = ctx.enter_context(tc.tile_pool(name="q", bufs=2))
    cx.dq = ctx.enter_context(tc.tile_pool(name="dq", bufs=2))
    cx.amax = ctx.enter_context(tc.tile_pool(name="amax", bufs=3))
    cx.scale = ctx.enter_context(tc.tile_pool(name="scale", bufs=2))
    cx.xT = ctx.enter_context(tc.tile_pool(name="xT", bufs=MTILES))
    cx.wT = ctx.enter_context(tc.tile_pool(name="wT", bufs=2))
    cx.mpsum = ctx.enter_context(tc.tile_pool(name="mpsum", bufs=6, space="PSUM"))
    cx.tpsum = ctx.enter_context(tc.tile_pool(name="tpsum", bufs=2, space="PSUM"))
    cx.evac = ctx.enter_context(tc.tile_pool(name="evac", bufs=3))
    cx.const = ctx.enter_context(tc.tile_pool(name="const", bufs=1))
    cx.dram_small = ctx.enter_context(
        tc.tile_pool(name="scratch_s", bufs=8, space="DRAM")
    )
    cx.xT_units = [None] * MTILES

    cx.ident_f32 = cx.const.tile([P, P], F32, tag="ident")
    make_identity(nc, cx.ident_f32)

    # ones row [1, 128] bf16 for the bias k-slice of xT
    cx.ones = cx.const.tile([1, P], BF16, tag="ones")
    nc.scalar.memzero(cx.ones[:])
    nc.scalar.add(cx.ones[:], cx.ones[:], 1.0)

    # bias as bf16 in DRAM scratch, laid out [32, 128] row-major = b[4096]
    bt = cx.const.tile([32, P], F32, tag="bt")
    nc.sync.dma_start(bt[:], b)
    btb = cx.const.tile([32, P], BF16, tag="btb")
    nc.vector.tensor_copy(btb[:], bt[:])
    bias_dram = cx.dram_small.tile([32, P], BF16, tag="bias_dram")
    nc.gpsimd.dma_start(bias_dram[:], btb[:])
    cx.bias_bf16 = bias_dram

    # ---- production + sweeps ----
    # w row tiles stream through a 2-stage pipeline (scale stage one row
    # tile ahead of qdq stage) woven between sweeps; x units fill during
    # panel 0.
    wT0 = _alloc_wT(cx, nc, 0)
    staged = {}
    for i in range(WR):
        staged[i] = _w_scale_stage(cx, nc, w, i)
        if i > 0:
            _w_qdq_stage(cx, nc, staged.pop(i - 1), i - 1, wT0)
    _w_qdq_stage(cx, nc, staged.pop(WR - 1), WR - 1, wT0)

    wT1 = _alloc_wT(cx, nc, 1)
    for mi in range(MTILES):
        _emit_x_unit(cx, nc, x, mi)
        # weave w panel-1 row tiles early: qdq of the staged tile first
        # (its bounce flew during this x unit's amax), then the next scale
        # stage -- keeps at most 2 row tiles of nat halves live.
        if 1 <= mi <= WR:
            _w_qdq_stage(cx, nc, staged.pop(mi - 1), mi - 1, wT1)
        if mi < WR:
            staged[mi] = _w_scale_stage(cx, nc, w, WR + mi)
        _emit_sweep(cx, nc, out, mi, 0, wT0)

    # Steady state: panel p sweeps while panel p+1's row tiles run the
    # 2-stage pipeline in the first 5 sweep slots.
    wTs = {1: wT1}
    for pn in range(1, PANELS):
        nxt = pn + 1
        if nxt < PANELS:
            wTs[nxt] = _alloc_wT(cx, nc, nxt)
        for mi in range(MTILES):
            if nxt < PANELS:
                if mi < WR:
                    staged[mi] = _w_scale_stage(cx, nc, w, nxt * WR + mi)
                if 1 <= mi <= WR:
                    _w_qdq_stage(cx, nc, staged.pop(mi - 1), mi - 1, wTs[nxt])
            _emit_sweep(cx, nc, out, mi, pn, wTs[pn])
    return


def build_core_bass():
    nc = bacc.Bacc(
        "TRN2", target_bir_lowering=False, debug=False, num_devices=N_CORES
    )
    x = nc.dram_tensor("x", [M_SH, K], F32, kind="ExternalInput").ap()
    w = nc.dram_tensor("w", [N, K], F32, kind="ExternalInput").ap()
    b = nc.dram_tensor("b", [32, P], F32, kind="ExternalInput").ap()
    out = nc.dram_tensor("out", [M_SH, N], BF16, kind="ExternalOutput").ap()
    with tile.TileContext(nc) as tc:
        with ExitStack() as stack:
            tc.ctx = stack
            fp8_linear_core_kernel(tc, out, x, w, b)
    nc.compile()
    return nc


_NC_CACHE = []


def _get_nc():
    if not _NC_CACHE:
        _NC_CACHE.append(build_core_bass())
    return _NC_CACHE[0]


def kernel(x, weight, bias):
    """Full-problem entry point: x [2,4096,4096] f32, weight [4096,4096] f32,
    bias [4096] f32 -> [2,4096,4096] f32."""
    from concourse.bass_utils import run_bass_kernel_spmd

    x2d = np.ascontiguousarray(x.reshape(M, K), dtype=np.float32)
    weight = np.ascontiguousarray(weight, dtype=np.float32)
    b32 = np.ascontiguousarray(bias.reshape(32, P), dtype=np.float32)

    nc = _get_nc()

    in_maps = []
    for core in range(N_CORES):
        in_maps.append(
            {
                "x": np.ascontiguousarray(x2d[core * M_SH : (core + 1) * M_SH]),
                "w": weight,
                "b": b32,
            }
        )

    res = run_bass_kernel_spmd(nc, in_maps, core_ids=list(range(N_CORES)))
    global LAST_EXEC_TIME_NS
    LAST_EXEC_TIME_NS = res.exec_time_ns

    out = np.empty((M, N), dtype=np.float32)
    for core in range(N_CORES):
        out[core * M_SH : (core + 1) * M_SH] = np.asarray(
            res.results[core]["out"]
        ).astype(np.float32)
    return out.reshape(B, S, N)
